# revision 1
# baseline (speedup 1.0000x reference)
"""AttentionAugmentedConv on 8 Trainium2 cores — data-parallel over batch (1 image/core).

Per-core plan (image = 32x32x256, flat i = x*32+y, 1024 positions):
  - xT_pad34: transposed image [c, (x+1)*34+(y+1)] with zero halo, fp16.
    Conv taps become plain shifted-window APs (zero padding baked in).
  - conv (out^T [f,i] form) + qkv projections on the tensor engine, fp16.
  - Relative-position logits folded into the QK matmul by augmenting
    q/k with 64 extra dims: q_aug = [q, Rw_gathered, Rh_gathered],
    k_aug = [k, onehot_w(yk), onehot_h(xk)].  Rw/Rh are gathered from the
    per-head rel-logit matmul R = q @ key_rel^T via a DRAM round trip
    (the per-partition shifted window needs a flat-address AP -> DRAM).
  - S^T = k_aug^T.T @ q_aug^T computed with j on partitions so softmax
    needs no transposes: exp on ScalarE (no max subtraction; logits are
    bounded, bias -12 keeps fp16 exp in range), denominator + P@V fused
    via vext = [v | 1] with 9-wide per-head blocks -> one pv_all PSUM
    tile [72, 1024] holding nums (rows 9n+d) and dens (rows 9n+8).
  - divide: recip [72,512] + per-head partition_broadcast + one mul ->
    attnALL [72, 1024]; output projection is 1 matmul per i-tile against
    attn_w padded with zero rows at 9n+8.
  - conv is emitted AFTER attention so its matmuls fill PE gaps during
    the ACT(exp)-bound attention phase.
"""

import sys
from contextlib import ExitStack

import numpy as np

if "/opt/trn_rl_repo" not in sys.path:
    sys.path.insert(0, "/opt/trn_rl_repo")

import concourse.bacc as bacc
import concourse.mybir as mybir
import concourse.tile as tile
from concourse import bass_utils

F16 = mybir.dt.float16
F32 = mybir.dt.float32
F8E4 = mybir.dt.float8e4
F8E5 = mybir.dt.float8e5
AF = mybir.ActivationFunctionType

NH, DKH, DVH = 8, 8, 8
H = W = 32
HW = H * W
CIN = 256
FCONV = 192  # conv output channels
EXP_BIAS = -8.0

TRACE = False
LAST_EXEC_NS = None
LAST_RESULTS = None

_cache = {}
import os
SKIP = set(os.environ.get("KSKIP", "").split(",")) - {""}


def _build(loop=None):
    nc = bacc.Bacc("TRN2", target_bir_lowering=False, debug=False)
    names = {}
    ctx = ExitStack()
    tc = ctx.enter_context(tile.TileContext(nc))

    dram = ctx.enter_context(tc.tile_pool(name="dram", bufs=1, space="DRAM"))
    x_d = dram.tile([HW, CIN], F16, kind="ExternalInput", name="x", tag="x")
    cw_d = dram.tile([9, 2, 128, FCONV], F16, kind="ExternalInput", name="cw", tag="cw")
    wkq_d = dram.tile([2, 128, 128], F16, kind="ExternalInput", name="wkq", tag="wkq")
    wqv_d = dram.tile([2, 128, 128], F16, kind="ExternalInput", name="wqv", tag="wqv")
    krel_d = dram.tile([128, 1008], F8E4, kind="ExternalInput", name="krel", tag="krel")
    oh_d = dram.tile([64, HW], F8E4, kind="ExternalInput", name="onehot", tag="onehot")
    id_d = dram.tile([128, 128], F16, kind="ExternalInput", name="ident", tag="ident")
    aw_d = dram.tile([8, 512], F16, kind="ExternalInput", name="attnw", tag="attnw")
    out_d = dram.tile([HW, 256], F32, kind="ExternalOutput", name="out", tag="out")
    r_ds = [dram.tile([128, 1008], F8E4, name=f"rscratch{i}", tag=f"rscratch{i}")
            for i in range(8)]

    names.update(x=x_d.name, cw=cw_d.name, wkq=wkq_d.name, wqv=wqv_d.name,
                 krel=krel_d.name, onehot=oh_d.name, ident=id_d.name,
                 attnw=aw_d.name, out=out_d.name)

    const = ctx.enter_context(tc.tile_pool(name="const", bufs=1))
    sb = ctx.enter_context(tc.tile_pool(name="sb", bufs=1))
    pipe = ctx.enter_context(tc.tile_pool(name="pipe", bufs=3))
    rpipe = ctx.enter_context(tc.tile_pool(name="rpipe", bufs=4))
    expp = ctx.enter_context(tc.tile_pool(name="expp", bufs=8))
    denp = ctx.enter_context(tc.tile_pool(name="denp", bufs=4))

    pA = ctx.enter_context(tc.tile_pool(name="pA", bufs=2, space="PSUM"))
    pS = ctx.enter_context(tc.tile_pool(name="pS", bufs=2, space="PSUM"))
    pPV = ctx.enter_context(tc.tile_pool(name="pPV", bufs=2, space="PSUM"))

    def emit():
        id_sb = const.tile([128, 128], F16, name="ident", tag="ident")
        nc.sync.dma_start(id_sb[:], id_d[:])

        # ---- x: ONE DMA into [128, 2048] (it-major blocks of 256 chans) ----
        x_sb = sb.tile([128, 8 * CIN], F16, name="x_in", tag="x_in")
        xs = x_d[:].copy()
        xs.ap = mybir.VecI64Pair([[CIN, 128], [128 * CIN, 8], [1, CIN]])
        xs.offset = 0
        nc.sync.dma_start(x_sb[:], xs)

        # ---- x -> xT_pad (transposed, padded layout) ----
        PADW = 1228
        PADO = 36
        xTp = [sb.tile([128, PADW], F16, name=f"xTp{cb}", tag=f"xTp{cb}") for cb in range(2)]
        for cb in range(2):
            nc.gpsimd.memset(xTp[cb][:], 0.0)

        def xwin(cb, a0, na, b0, nb):
            """stationary-side window AP [128c, na, nb] at padded rows a0.., cols b0.."""
            w = xTp[cb][:].copy()
            w.ap = mybir.VecI64Pair([[PADW, 128], [34, na], [1, nb]])
            w.offset = PADO + a0 * 34 + b0
            return w

        def pslice(cb, r0, nr, delta=0):
            """moving-side contiguous slice covering padded rows [r0, r0+nr) shifted by delta"""
            s = PADO + 34 * r0 + delta
            return xTp[cb][:, s:s + 34 * nr]

        for it in range(8):
            for cb in range(2):
                ps = pA.tile([128, 128], F16, name="pAt", tag="pA")
                nc.tensor.transpose(ps[:], x_sb[:, 256 * it + 128 * cb:256 * it + 128 * (cb + 1)],
                                    id_sb[:])
                nc.vector.tensor_copy(xwin(cb, 4 * it + 1, 4, 1, 32), ps[:])

        # ---- weights to SBUF (merged DMAs) ----
        wkq_sb = const.tile([128, 256], F16, name="wkq", tag="wkq")
        wqv_sb = const.tile([128, 256], F16, name="wqv", tag="wqv")
        for t_sb, t_d in ((wkq_sb, wkq_d), (wqv_sb, wqv_d)):
            s = t_d[:].copy()
            s.ap = mybir.VecI64Pair([[128, 128], [128 * 128, 2], [1, 128]])
            s.offset = 0
            nc.sync.dma_start(t_sb[:], s)
        krel_sb = const.tile([128, 1008], F8E4, name="krel", tag="krel")
        nc.sync.dma_start(krel_sb[:], krel_d[:])
        bias_sb = const.tile([128, 1], F32, name="expbias", tag="expbias")
        nc.vector.memset(bias_sb[:], EXP_BIAS)

        # row-aligned output chunks over the padded grid: rows [0,12),[12,24),[24,34)
        CHUNKS = ((0, 12, 1, 12), (12, 12, 12, 24), (24, 10, 24, 33))

        # ---- kqvT: [k(64); q_scaled(64)] x 1024 dense, via padded chunks ----
        kqvT = sb.tile([128, HW], F8E4, name="kqvT", tag="kqvT")
        for (r0, nr, alo, ahi) in CHUNKS:
            ps = pA.tile([128, 34 * 12], F32, name="pA", tag="pA")
            psl = ps[:, 0:34 * nr]
            for cb in range(2):
                nc.tensor.matmul(psl, wkq_sb[:, 128 * cb:128 * (cb + 1)], pslice(cb, r0, nr),
                                 start=(cb == 0), stop=(cb == 1))
            pv3 = ps[:, 0:34 * nr].rearrange("p (a b) -> p a b", a=nr, b=34)
            nc.vector.tensor_copy(kqvT[:, 32 * (alo - 1):32 * (ahi - 1)],
                                  pv3[:, alo - r0:ahi - r0, 1:33])
        # ---- qvT dense, same trick ----
        qvT = sb.tile([128, HW], F16, name="qvT", tag="qvT")
        for (r0, nr, alo, ahi) in CHUNKS:
            ps = pA.tile([128, 34 * 12], F32, name="pA", tag="pA")
            psl = ps[:, 0:34 * nr]
            for cb in range(2):
                nc.tensor.matmul(psl, wqv_sb[:, 128 * cb:128 * (cb + 1)], pslice(cb, r0, nr),
                                 start=(cb == 0), stop=(cb == 1))
            pv3 = ps[:, 0:34 * nr].rearrange("p (a b) -> p a b", a=nr, b=34)
            nc.vector.tensor_copy(qvT[:, 32 * (alo - 1):32 * (ahi - 1)],
                                  pv3[:, alo - r0:ahi - r0, 1:33])

        # ---- k_aug^T all heads, fp8 pair-padded: [72, 8 * (2*1024)]
        # head n cols: 2048n + ko*1024 + j ; ko=1 half is zeros ----
        kaugT = sb.tile([72, NH * 2 * HW], F8E4, name="kaugT", tag="kaugT")
        for n in range(NH):
            nc.gpsimd.memset(kaugT[:, 2 * HW * n + HW:2 * HW * (n + 1)], 0.0)
        for n in range(NH):
            nc.sync.dma_start(kaugT[0:8, 2 * HW * n:2 * HW * n + HW],
                              kqvT[8 * n:8 * (n + 1), :])
        ohs = oh_d[:].copy()
        ohs.ap = mybir.VecI64Pair([[HW, 64], [0, NH], [1, HW]])
        ohs.offset = 0
        od = kaugT[8:72, :].copy()
        od.ap = mybir.VecI64Pair([[NH * 2 * HW, 64], [2 * HW, NH], [1, HW]])
        od.offset = od.offset
        nc.scalar.dma_start(od, ohs)

        # ---- qaug (q | Rw | Rh per head) + vext ([v|1|eps] 32-wide blocks) ----
        qaug = [sb.tile([128, NH * 72], F16, name=f"qaug{it}", tag=f"qaug{it}") for it in range(8)]
        vext = [sb.tile([128, 2 * NH * 32], F8E5, name=f"vext{jp}", tag=f"vext{jp}") for jp in range(4)]
        for it in range(8):
            ps = pA.tile([128, 128], F16, name="pAt", tag="pA")
            nc.tensor.transpose(ps[:], qvT[:, 128 * it:128 * (it + 1)], id_sb[:])
            qa = qaug[it][:].rearrange("p (n d) -> p n d", n=NH, d=72)[:, :, 0:8]
            nc.vector.tensor_copy(qa, ps[:, 0:64].rearrange("p (n d) -> p n d", n=NH, d=8))
            vh = vext[it // 2][:, 256 * (it % 2):256 * (it % 2) + 256]
            va = vh.rearrange("p (n d) -> p n d", n=NH, d=32)[:, :, 0:8]
            nc.vector.tensor_copy(va, ps[:, 64:128].rearrange("p (n d) -> p n d", n=NH, d=8))
            ones = vh.rearrange("p (n d) -> p n d", n=NH, d=32)[:, :, 8:9]
            nc.vector.memset(ones, 1.0)
            eps = vh.rearrange("p (n d) -> p n d", n=NH, d=32)[:, :, 9:32]
            nc.vector.memset(eps, 1e-4)

        # ---- R = q @ krel_blockdiag, all heads packed; stage via DRAM ----
        if 'rel' in SKIP:
            for it in range(8):
                nc.gpsimd.memset(qaug[it][:, 64:576], 0.0)
        copy_engs = [nc.scalar.copy, nc.vector.tensor_copy]
        for it in range(8 if 'rel' not in SKIP else 0):
            rsb = rpipe.tile([128, 1008], F8E4, name=f"rsb{it}", tag=f"rsb{it}", bufs=1)
            for mh in range(2):
                ps = pA.tile([128, 504], F32, name="pR", tag="pA")
                nc.tensor.matmul(ps[:],
                                 kqvT[64:128, 128 * it:128 * (it + 1)],
                                 krel_sb[64:128, 504 * mh:504 * (mh + 1)],
                                 start=True, stop=True)
                copy_engs[(2 * it + mh) % 2](rsb[:, 504 * mh:504 * (mh + 1)], ps[:])
            nc.sync.dma_start(r_ds[it][:], rsb[:])

        # gather (both halves from DRAM, mixed-stride flat APs):
        #   qint[p, yk*8+n]     = Rw[p, (31-y(p)+yk)*8+n]
        #   qint[p, 256+xk*8+n] = Rh[p, (31-x(p)+xk)*8+n]   (x = 4*it + p//32)
        qint = [sb.tile([128, 512], F8E4, name=f"qint{it}", tag=f"qint{it}") for it in range(8)]
        dma_engs = [nc.sync, nc.scalar]
        for it in range(8 if 'rel' not in SKIP else 0):
            base = r_ds[it][:]
            gw = base.copy()
            gw.ap = mybir.VecI64Pair([[32 * 1008, 4], [1000, 32], [1, 256]])
            gw.offset = 31 * 8
            dma_engs[it % 2].dma_start(qint[it][:, 0:256], gw)
            gh = base.copy()
            gh.ap = mybir.VecI64Pair([[32 * 1008 - 8, 4], [1008, 32], [1, 256]])
            gh.offset = 504 + (31 - 4 * it) * 8
            dma_engs[(it + 1) % 2].dma_start(qint[it][:, 256:512], gh)
        # de-interleave: qaug[it][:, 72n+8+32t+yk] = qint[it][:, 256t+yk*8+n]
        for it in range(8 if 'rel' not in SKIP else 0):
            for t in range(2):
                o = qaug[it][:].copy()
                o.ap = mybir.VecI64Pair([[NH * 72, 128], [72, NH], [1, 32]])
                o.offset = 8 + 32 * t
                i = qint[it][:].copy()
                i.ap = mybir.VecI64Pair([[512, 128], [1, NH], [8, 32]])
                i.offset = 256 * t
                nc.vector.tensor_copy(o, i)

        # ---- transpose q_aug -> qaugT[n] [72, 2*1024] fp8, ko=1 zeros ----
        qaugT = [sb.tile([72, 2 * HW], F8E4, name=f"qaugT{n}", tag=f"qaugT{n}") for n in range(NH)]
        for n in range(NH):
            nc.gpsimd.memset(qaugT[n][:, HW:2 * HW], 0.0)
            for it in range(8):
                ps = pA.tile([72, 128], F16, name="pAt", tag="pA")
                nc.tensor.transpose(ps[:], qaug[it][:, 72 * n:72 * n + 72], id_sb[:])
                nc.vector.tensor_copy(qaugT[n][:, 128 * it:128 * (it + 1)], ps[:])

        # ---- attention: S^T(fp8 DoubleRow) -> exp(fp8e5) -> PV(fp8 DoubleRow,
        # per-head [32,512] tiles at base 0; den row 8) -> divide per head ----
        run_attn = 'attn' not in SKIP
        attnH = [sb.tile([9, HW], F16, name=f"attnH{n}", tag=f"attnH{n}")
                 for n in range(NH)]
        for n in range(NH if run_attn else 0):
            expS = []
            for jt in range(8):
                ps = pS.tile([128, HW], F32, name="pS", tag="pS")
                for h2 in range(2):
                    lh = kaugT[:].copy()
                    lh.ap = mybir.VecI64Pair([[NH * 2 * HW, 72], [HW, 2], [1, 128]])
                    lh.offset = 2 * HW * n + 128 * jt
                    rh = qaugT[n][:].copy()
                    rh.ap = mybir.VecI64Pair([[2 * HW, 72], [HW, 2], [1, 512]])
                    rh.offset = 512 * h2
                    nc.tensor.matmul(ps[:, 512 * h2:512 * (h2 + 1)], lh, rh,
                                     start=True, stop=True,
                                     perf_mode=mybir.MatmulPerfMode.DoubleRow)
                if jt % 2 == 0:
                    es2 = expp.tile([128, 2 * HW], F8E5, name="expS", tag="expS")
                    expS.append(es2)
                nc.scalar.activation(expS[jt // 2][:, HW * (jt % 2):HW * (jt % 2) + HW],
                                     ps[:], AF.Exp, bias=bias_sb[:], scale=1.0)
            for h2 in range(2 if 'pv' not in SKIP else 0):
                pv = pPV.tile([32, 512], F32, name="pv", tag="pv")
                for jp in range(4):
                    lh = vext[jp][:].copy()
                    lh.ap = mybir.VecI64Pair([[2 * NH * 32, 128], [NH * 32, 2], [1, 32]])
                    lh.offset = 32 * n
                    rh = expS[jp][:].copy()
                    rh.ap = mybir.VecI64Pair([[2 * HW, 128], [HW, 2], [1, 512]])
                    rh.offset = 512 * h2
                    nc.tensor.matmul(pv[:], lh, rh, start=(jp == 0), stop=(jp == 3),
                                     perf_mode=mybir.MatmulPerfMode.DoubleRow)
                if 'div' in SKIP:
                    continue
                denr = denp.tile([9, 512], F32, name="denr", tag="denr")
                nc.vector.reciprocal(denr[:], pv[0:9, :])
                denb = denp.tile([8, 512], F32, name="denb", tag="denb")
                sBC = denr[:].copy()
                sBC.ap = mybir.VecI64Pair([[512, 1], [0, 8], [1, 512]])
                sBC.offset = 8 * 512
                nc.sync.dma_start(denb[:], sBC)
                nc.vector.tensor_mul(attnH[n][0:8, 512 * h2:512 * (h2 + 1)],
                                     pv[0:8, :], denb[:])

        # ---- conv weights (ONE DMA) + conv branch (after attention: fills
        # PE gaps in the ACT-bound attention phase) ----
        cw_all = const.tile([128, 18 * FCONV], F16, name="cwall", tag="cwall")
        cs = cw_d[:].copy()
        cs.ap = mybir.VecI64Pair([[FCONV, 128], [128 * FCONV, 18], [1, FCONV]])
        cs.offset = 0
        nc.scalar.dma_start(cw_all[:], cs)
        awF_sb = const.tile([8, 512], F16, name="attnw", tag="attnw")
        nc.scalar.dma_start(awF_sb[:], aw_d[:])

        convT = [sb.tile([128, HW], F16, name="convT0", tag="convT0"),
                 sb.tile([64, HW], F16, name="convT1", tag="convT1")]
        for ft, fm in (() if 'conv' in SKIP else ((0, 128), (1, 64))):
            for (r0, nr, alo, ahi) in CHUNKS:
                ps = pA.tile([128, 34 * 12], F32, name="pC", tag="pA")
                psl = ps[0:fm, 0:34 * nr]
                k = 0
                for t in range(9):
                    dx, dy = divmod(t, 3)
                    for cb in range(2):
                        nc.tensor.matmul(psl,
                                         cw_all[:, (2 * t + cb) * FCONV + 128 * ft:
                                                (2 * t + cb) * FCONV + 128 * ft + fm],
                                         pslice(cb, r0, nr, (dx - 1) * 34 + (dy - 1)),
                                         start=(k == 0), stop=(k == 17))
                        k += 1
                pv3 = psl.rearrange("p (a b) -> p a b", a=nr, b=34)
                nc.vector.tensor_copy(convT[ft][0:fm, 32 * (alo - 1):32 * (ahi - 1)],
                                      pv3[:, alo - r0:ahi - r0, 1:33])

        # ---- assemble output: conv detranspose + attn projection ----
        for it in range(8):
            osb = pipe.tile([128, 256], F32, name="osb", tag="osb")
            for ft, fm in (() if 'conv' in SKIP else ((0, 128), (1, 64))):
                ps = pA.tile([128, 128], F16, name="pAt", tag="pA")
                nc.tensor.transpose(ps[0:128, 0:fm], convT[ft][0:fm, 128 * it:128 * (it + 1)],
                                    id_sb[0:fm, 0:fm])
                nc.vector.tensor_copy(osb[:, 128 * ft:128 * ft + fm], ps[0:128, 0:fm])
            if run_attn and 'pv' not in SKIP and 'div' not in SKIP:
                ps = pA.tile([128, 64], F32, name="pA", tag="pA")
                for n in range(NH):
                    nc.tensor.matmul(ps[:], attnH[n][0:8, 128 * it:128 * (it + 1)],
                                     awF_sb[0:8, 64 * n:64 * (n + 1)],
                                     start=(n == 0), stop=(n == NH - 1))
                nc.vector.tensor_copy(osb[:, 192:256], ps[:])
            nc.sync.dma_start(out_d[128 * it:128 * (it + 1), :], osb[:])

    if loop is None:
        emit()
    else:
        with tc.For_i(0, loop, 1):
            emit()

    ctx.close()
    nc.compile()
    return nc, names


def _prep(conv_w, qkv_w, attn_w, key_rel_w, key_rel_h):
    cw = np.ascontiguousarray(conv_w.reshape(9, 256, FCONV).reshape(9, 2, 128, FCONV)
                              .astype(np.float16))
    s = np.float32(DKH) ** -0.5
    wkq = qkv_w[:, 0:128].copy()
    wkq[:, 64:128] *= s
    wkq = np.ascontiguousarray(wkq.reshape(2, 128, 128).astype(np.float16))
    wqv = qkv_w[:, 64:192].copy()
    wqv[:, 0:64] *= s
    wqv = np.ascontiguousarray(wqv.reshape(2, 128, 128).astype(np.float16))
    kr = np.concatenate([key_rel_w.T, key_rel_h.T], axis=1)  # [8, 126]
    import ml_dtypes
    krel = np.zeros((128, 1008), ml_dtypes.float8_e4m3)
    m = np.arange(126)
    for n in range(8):
        krel[64 + 8 * n:72 + 8 * n, m * 8 + n] = kr.astype(ml_dtypes.float8_e4m3)
    oh = np.zeros((64, HW), ml_dtypes.float8_e4m3)
    j = np.arange(HW)
    oh[j % 32, j] = 1.0
    oh[32 + j // 32, j] = 1.0
    ident = np.eye(128, dtype=np.float16)
    aw2 = np.zeros((8, 512), np.float16)
    for n in range(8):
        aw2[:, 64 * n:64 * (n + 1)] = attn_w[8 * n:8 * (n + 1), :]
    return dict(cw=cw, wkq=wkq, wqv=wqv, krel=krel, onehot=oh, ident=ident,
                attnw=aw2)


def kernel(x, conv_w, conv_b, qkv_w, qkv_b, attn_w, attn_b, key_rel_w, key_rel_h):
    global LAST_EXEC_NS, LAST_RESULTS
    x = np.asarray(x, np.float32)
    B = x.shape[0]
    if "nc" not in _cache:
        _cache["nc"], _cache["names"] = _build()
    nc, names = _cache["nc"], _cache["names"]

    shared = _prep(np.asarray(conv_w, np.float32), np.asarray(qkv_w, np.float32),
                   np.asarray(attn_w, np.float32), np.asarray(key_rel_w, np.float32),
                   np.asarray(key_rel_h, np.float32))
    xf = x.reshape(B, HW, CIN).astype(np.float16)
    in_maps = []
    for b in range(B):
        m = {names[k]: v for k, v in shared.items()}
        m[names["x"]] = np.ascontiguousarray(xf[b])
        in_maps.append(m)

    res = bass_utils.run_bass_kernel_spmd(nc, in_maps, core_ids=list(range(B)),
                                          trace=TRACE)
    LAST_EXEC_NS = res.exec_time_ns
    LAST_RESULTS = res
    out = np.stack([res.results[b][names["out"]] for b in range(B)])
    return out.reshape(B, H, W, 256).astype(np.float32)

